# revision 40
# baseline (speedup 1.0000x reference)
"""Trainium2 Bass kernel for nn_AttentionHead (softmax over query axis).

Sharding (8 cores = 4 batches x 2): core pair (2b, 2b+1) handles batch b.
Rank h = c%2 owns KEY blocks of parity h: local chunk sb <-> global key
block gk = 2*sb + h.  Each core projects q for ALL 2048 rows (redundant,
avoids a mid-kernel AllGather) and k/v for its own 1024 rows only.

Per core (single SPMD program; h only appears in host-staged data):
  - host stages xT = x.T in bf16, columns [my 1024 rows | other 1024 rows]
    -> projections need no PE transposes and no gather
  - a tiny dummy collective issued at t~0 absorbs the collective-stream
    init barrier (~21.7us start + ~17us barrier + ~11.5us first-op cost)
    so the real collectives at the end pay only ~1.2us trigger latency
  - scores sT[s, t] = kb.T @ qT for queries t >= key block; causal via
    ADDITIVE masks on psum before exp (diag tri / first-remote-chunk)
  - Z[s] = sum_t E[s, t] is fully local (key-sharded) -> no AllReduce
  - v' = v/Z; AV accumulates zT[d, t] partials over local key blocks into
    4 PSUM banks; the low-column banks finish at sb=3 and ReduceScatter-A
    (bf16, pair, add) ships them while sb=4..7 still compute; RS-B ships
    the rest.  RS splits the D dim: core h ends with z[d in 64h..64h+64].
Host assembles the 8 [64, 2048] bf16 outputs into [4, 2048, 128] f32.
"""
import sys

for _p in ("/opt/trn_rl_repo",):
    if _p not in sys.path:
        sys.path.append(_p)

import numpy as np
import ml_dtypes

import concourse.bass as bass
import concourse.mybir as mybir
import concourse.tile as tile
from concourse import bacc
from concourse.bass import ds, ts
from concourse.bass_utils import run_bass_kernel_spmd
from concourse.masks import make_identity

BF16 = mybir.dt.bfloat16
F32 = mybir.dt.float32
U32 = mybir.dt.uint32
AF = mybir.ActivationFunctionType
ALU = mybir.AluOpType
AX = mybir.AxisListType

B, T, E, D = 4, 2048, 2048, 128
NLC = 8          # local 128-chunks per core (keys); queries = 2*NLC chunks
NE = 16          # E chunks of 128
SCALE = 1.0 / np.sqrt(D)
N_CORES = 8
REPLICA_GROUPS = [[0, 1], [2, 3], [4, 5], [6, 7]]
NEG = -1.0e30


def pieces(sb):
    """Column pieces [c0, width) of the valid query range [sb*128, 1024),
    split at absolute column 512 (PSUM-bank aligned)."""
    lo = sb * 128
    if lo < 512:
        return [(lo, 512 - lo), (512, 512)]
    return [(lo, 1024 - lo)]


def build_nc():
    nc = bacc.Bacc("TRN2", target_bir_lowering=False, debug=False,
                   num_devices=N_CORES)
    xT = nc.dram_tensor("xT", [E, T], BF16, kind="ExternalInput")
    wq = nc.dram_tensor("wq", [128, NE, D], BF16, kind="ExternalInput")
    wk = nc.dram_tensor("wk", [128, NE, D], BF16, kind="ExternalInput")
    wv = nc.dram_tensor("wv", [128, NE, D], BF16, kind="ExternalInput")
    dmask = nc.dram_tensor("dmask", [128, 128], BF16, kind="ExternalInput")
    rmask = nc.dram_tensor("rmask", [128, 128], BF16, kind="ExternalInput")
    hoff = nc.dram_tensor("hoff", [1, 5], U32, kind="ExternalInput")
    out = nc.dram_tensor("out", [64, T], BF16, kind="ExternalOutput")

    with tile.TileContext(nc) as tc:
        _body(nc, tc, xT, wq, wk, wv, dmask, rmask, hoff, out)
    nc.compile()
    return nc


def _body(nc, tc, xT, wq, wk, wv, dmask, rmask, hoff, out):
    with (
        tc.tile_pool(name="const", bufs=1) as const_pool,
        tc.tile_pool(name="dram", bufs=1, space="DRAM") as dram_pool,
        tc.tile_pool(name="proj", bufs=1) as proj_pool,
        tc.tile_pool(name="escore", bufs=1) as e_pool,
    ):
        # ---- constants / weights (SWDGE: keep HWDGE queues free for x) ----
        # ---- dummy early collective: absorbs cc-stream init + barrier ----
        dummy_sb = const_pool.tile([128, 16], BF16, name="dummy_sb")
        nc.vector.memset(dummy_sb[:], 0.0)
        dummy_in = dram_pool.tile([128, 16], BF16, name="dummy_in")
        dummy_out = dram_pool.tile([2, 128, 16], BF16, name="dummy_out")
        nc.gpsimd.dma_start(out=dummy_in[:], in_=dummy_sb[:])
        nc.gpsimd.collective_compute(
            "AllGather", ALU.bypass, replica_groups=REPLICA_GROUPS,
            ins=[dummy_in[:].opt()], outs=[dummy_out[:].opt()],
        )

        ident = const_pool.tile([128, 128], BF16, name="ident")
        make_identity(nc, ident)
        wq_sb = const_pool.tile([128, NE, D], BF16, name="wq_sb")
        wk_sb = const_pool.tile([128, NE, D], BF16, name="wk_sb")
        wv_sb = const_pool.tile([128, NE, D], BF16, name="wv_sb")
        nc.sync.dma_start(out=wk_sb[:], in_=wk[:])
        nc.scalar.dma_start(out=wq_sb[:], in_=wq[:])
        nc.gpsimd.dma_start(out=wv_sb[:], in_=wv[:])
        dmask_sb = const_pool.tile([128, 128], BF16, name="dmask_sb")
        rmask_sb = const_pool.tile([128, 128], BF16, name="rmask_sb")
        nc.gpsimd.dma_start(out=dmask_sb[:], in_=dmask[:])
        nc.gpsimd.dma_start(out=rmask_sb[:], in_=rmask[:])
        stats = const_pool.tile([128, NLC * 4], F32, name="stats")
        nc.vector.memset(stats[:], 0.0)

        # runtime pair-rank offsets: hoff = [h*1024, (1-h)*1024, h*512,
        # (1-h)*512, h*64]
        _r = [nc.sync.alloc_register(f"rv{i}") for i in range(5)]
        for i in range(5):
            nc.sync.reg_load(_r[i], hoff[0:1, i:i + 1])
        rv_h = nc.sync.snap(_r[0], donate=True, min_val=0, max_val=1024)
        rv_r = nc.sync.snap(_r[1], donate=True, min_val=0, max_val=1024)
        rv_h5 = nc.sync.snap(_r[2], donate=True, min_val=0, max_val=512)
        rv_r5 = nc.sync.snap(_r[3], donate=True, min_val=0, max_val=512)
        rv64 = nc.sync.snap(_r[4], donate=True, min_val=0, max_val=64)
        _r2 = nc.scalar.alloc_register("rv64b")
        nc.scalar.reg_load(_r2, hoff[0:1, 4:5])
        rv64b = nc.scalar.snap(_r2, donate=True, min_val=0, max_val=64)

        # ---- x load (pre-transposed bf16, fully contiguous, e-ordered) ----
        x_sb = []
        for e in range(NE):
            t_x = const_pool.tile([128, T], BF16, name=f"xT{e}")
            nc.sync.dma_start(out=t_x[:, ds(0, 1024)],
                              in_=xT[ts(e, 128), ds(0, 1024)])
            nc.scalar.dma_start(out=t_x[:, ds(1024, 1024)],
                                in_=xT[ts(e, 128), ds(1024, 1024)])
            x_sb.append(t_x)

        qT_sb = proj_pool.tile([128, T], BF16, name="qT_sb")
        kT_sb = proj_pool.tile([128, NLC * 128], BF16, name="kT_sb")
        vT_sb = proj_pool.tile([128, NLC * 128], BF16, name="vT_sb")
        v_loc = proj_pool.tile([128, NLC * 128], BF16, name="v_loc")
        v_sc = proj_pool.tile([128, NLC * 128], BF16, name="v_sc")
        recip = const_pool.tile([128, NLC], F32, name="recip")

        # ---- k & q projections, interleaved per e-chunk ----
        with tc.tile_pool(name="pj_psum", bufs=1, space="PSUM") as pj:
            junk = pj.tile([128, 512], F32, tag="k0")
            for i in range(6):
                nc.tensor.matmul(junk[:], lhsT=wk_sb[:, i, :],
                                 rhs=wk_sb[:, ds(0, 4), :],
                                 start=True, stop=True)
            qp = [pj.tile([128, 512], F32, tag=f"p{i}", name=f"qp{i}")
                  for i in range(4)]
            k0 = pj.tile([128, 512], F32, tag="k0")
            k1 = pj.tile([128, 512], F32, tag="k1")
            for e in range(NE):
                st, sp = (e == 0), (e == NE - 1)
                nc.tensor.matmul(k0[:], lhsT=wk_sb[:, e, :],
                                 rhs=x_sb[e][:, ds(0, 512)], start=st, stop=sp)
                nc.tensor.matmul(k1[:], lhsT=wk_sb[:, e, :],
                                 rhs=x_sb[e][:, ds(512, 512)], start=st, stop=sp)
                for i in range(4):
                    nc.tensor.matmul(qp[i][:], lhsT=wq_sb[:, e, :],
                                     rhs=x_sb[e][:, ds(i * 512, 512)],
                                     start=st, stop=sp)
            nc.vector.tensor_copy(out=kT_sb[:, ds(0, 512)], in_=k0[:])
            nc.vector.tensor_copy(out=qT_sb[:, ds(0, 512)], in_=qp[0][:])
            nc.vector.tensor_copy(out=kT_sb[:, ds(512, 512)], in_=k1[:])
            for i in range(1, 4):
                nc.vector.tensor_copy(out=qT_sb[:, ds(i * 512, 512)], in_=qp[i][:])

        # ---- scores / exp / piece-sums; masks ride the PE as matmuls ----
        with (
            tc.tile_pool(name="vp_psum", bufs=1, space="PSUM") as vp,
            tc.tile_pool(name="sc_psum", bufs=2, space="PSUM") as scp,
            tc.tile_pool(name="zt_psum", bufs=1, space="PSUM") as ztp,
        ):
            # ---- v projection + v natural via PE transpose ----
            v0 = vp.tile([128, 512], F32, tag="v0")
            v1 = vp.tile([128, 512], F32, tag="v1")
            for e in range(NE):
                st, sp = (e == 0), (e == NE - 1)
                nc.tensor.matmul(v0[:], lhsT=wv_sb[:, e, :],
                                 rhs=x_sb[e][:, ds(0, 512)], start=st, stop=sp)
                nc.tensor.matmul(v1[:], lhsT=wv_sb[:, e, :],
                                 rhs=x_sb[e][:, ds(512, 512)], start=st, stop=sp)
            nc.vector.tensor_copy(out=vT_sb[:, ds(0, 512)], in_=v0[:])
            nc.vector.tensor_copy(out=vT_sb[:, ds(512, 512)], in_=v1[:])
            for sb in range(NLC):
                tpv = vp.tile([128, 128], BF16, tag="v0")
                nc.tensor.transpose(out=tpv[:], in_=vT_sb[:, ts(sb, 128)],
                                    identity=ident[:])
                nc.vector.tensor_copy(out=v_loc[:, ts(sb, 128)], in_=tpv[:])

            # ---- scores / exp / Z / v' / AV streamed per key block ----
            zt = [ztp.tile([128, 512], F32, tag=f"zt{j}", name=f"zt{j}")
                  for j in range(4)]
            zT_f = proj_pool.tile([128, 2048], BF16, name="zT_f")
            zinA = dram_pool.tile([128, 1024], BF16, name="zinA")
            zinB = dram_pool.tile([128, 1024], BF16, name="zinB")
            agA = dram_pool.tile([2, 128, 1024], BF16, name="agA")
            agB = dram_pool.tile([2, 128, 1024], BF16, name="agB")
            pA0 = proj_pool.tile([64, 1024], BF16, name="pA0")
            pA1 = proj_pool.tile([64, 1024], BF16, name="pA1")
            pB0 = proj_pool.tile([64, 1024], BF16, name="pB0")
            pB1 = proj_pool.tile([64, 1024], BF16, name="pB1")
            zoA = proj_pool.tile([64, 1024], BF16, name="zoA")
            zoB = proj_pool.tile([64, 1024], BF16, name="zoB")
            for sb in range(NLC):
                esb = e_pool.tile([128, 2048], BF16, name=f"esb{sb}")
                kb = kT_sb[:, ts(sb, 128)]
                for half in range(2):  # 0: my queries, 1: peer's queries
                    mask = dmask_sb if half == 0 else rmask_sb
                    for pi, (c0, pw) in enumerate(pieces(sb)):
                        sc = scp.tile([128, 512], F32, tag="sc")
                        nc.tensor.matmul(sc[:, ds(0, pw)], lhsT=kb,
                                         rhs=qT_sb[:, ds(half * 1024 + c0, pw)],
                                         start=True, stop=(pi != 0),
                                         skip_group_check=True)
                        if pi == 0:  # causal-boundary chunk: additive mask
                            # sc[s,t] += mask[t,s] (= mask.T @ I), on PE
                            nc.tensor.matmul(sc[:, ds(0, 128)], lhsT=mask[:],
                                             rhs=ident[:],
                                             start=False, stop=True,
                                             skip_group_check=True)
                        nc.scalar.activation(out=esb[:, ds(half * 1024 + c0, pw)],
                                             in_=sc[:, ds(0, pw)],
                                             func=AF.Exp, scale=SCALE)
                        nc.vector.reduce_sum(
                            out=stats[:, ds(sb * 4 + half * 2 + pi, 1)],
                            in_=esb[:, ds(half * 1024 + c0, pw)], axis=AX.X)
                zs = const_pool.tile([128, 1], F32, tag="zs")
                nc.vector.reduce_sum(out=zs[:], in_=stats[:, ds(sb * 4, 4)],
                                     axis=AX.X)
                nc.vector.reciprocal(out=recip[:, ds(sb, 1)], in_=zs[:])
                nc.vector.tensor_scalar_mul(
                    out=v_sc[:, ts(sb, 128)], in0=v_loc[:, ts(sb, 128)],
                    scalar1=recip[:, ds(sb, 1)])
                vs = v_sc[:, ts(sb, 128)]
                for half in range(2):
                    for (c0, pw) in pieces(sb):
                        j = half * 2 + (c0 // 512)
                        nc.tensor.matmul(
                            zt[j][:, ds(c0 % 512, pw)], lhsT=vs,
                            rhs=esb[:, ds(half * 1024 + c0, pw)],
                            start=(sb == 0),
                            stop=(sb == (3 if c0 < 512 else NLC - 1)),
                            skip_group_check=True)
                if sb == 3:
                    # low-column banks are complete: ship the first RS now
                    nc.vector.tensor_copy(out=zT_f[:, ds(0, 512)], in_=zt[0][:])
                    nc.vector.tensor_copy(out=zT_f[:, ds(1024, 512)],
                                          in_=zt[2][:])
                    nc.sync.dma_start(out=zinA[:, ds(rv_h5, 512)],
                                      in_=zT_f[:, ds(0, 512)])
                    nc.sync.dma_start(out=zinA[:, ds(rv_r5, 512)],
                                      in_=zT_f[:, ds(1024, 512)])
                    nc.gpsimd.collective_compute(
                        "AllGather", ALU.bypass, replica_groups=REPLICA_GROUPS,
                        ins=[zinA[:].opt()], outs=[agA[:].opt()],
                    )
                    nc.sync.dma_start(out=pA0[:],
                                      in_=agA[0, ds(rv64, 64), :])
                    nc.scalar.dma_start(out=pA1[:],
                                        in_=agA[1, ds(rv64b, 64), :])
                    nc.vector.tensor_tensor(out=zoA[:], in0=pA0[:],
                                            in1=pA1[:], op=ALU.add)
                    nc.sync.dma_start(out=out[:, ds(0, 1024)], in_=zoA[:])
            nc.vector.tensor_copy(out=zT_f[:, ds(512, 512)], in_=zt[1][:])
            nc.vector.tensor_copy(out=zT_f[:, ds(1536, 512)], in_=zt[3][:])
            nc.sync.dma_start(out=zinB[:, ds(rv_h5, 512)],
                              in_=zT_f[:, ds(512, 512)])
            nc.sync.dma_start(out=zinB[:, ds(rv_r5, 512)],
                              in_=zT_f[:, ds(1536, 512)])
            nc.gpsimd.collective_compute(
                "AllGather", ALU.bypass, replica_groups=REPLICA_GROUPS,
                ins=[zinB[:].opt()], outs=[agB[:].opt()],
            )
            nc.sync.dma_start(out=pB0[:], in_=agB[0, ds(rv64, 64), :])
            nc.scalar.dma_start(out=pB1[:], in_=agB[1, ds(rv64b, 64), :])
            nc.vector.tensor_tensor(out=zoB[:], in0=pB0[:], in1=pB1[:],
                                    op=ALU.add)
            nc.sync.dma_start(out=out[:, ds(1024, 1024)], in_=zoB[:])


_NC_CACHE = None


def _get_nc():
    global _NC_CACHE
    if _NC_CACHE is None:
        _NC_CACHE = build_nc()
    return _NC_CACHE


def _w_tiles(W):
    return np.ascontiguousarray(
        np.asarray(W, np.float32).reshape(NE, 128, D).transpose(1, 0, 2)
    ).astype(ml_dtypes.bfloat16)


def make_in_maps(x_in, Wq, Wk, Wv):
    x_in = np.asarray(x_in, np.float32)
    wqt, wkt, wvt = _w_tiles(Wq), _w_tiles(Wk), _w_tiles(Wv)
    # mask weights: score psum gets += maskW[t, s] via maskW.T @ I, so
    # maskW[e, s] = NEG where s > e adds NEG at [s, t] for t < s (diag),
    # and all-NEG adds NEG everywhere (invalid first remote chunk, h=1)
    triW = np.where(np.arange(128)[:, None] < np.arange(128)[None, :],
                    NEG, 0.0).astype(ml_dtypes.bfloat16)
    in_maps = []
    for c in range(N_CORES):
        b, h = c // 2, c % 2
        mine = np.concatenate(
            [np.arange((2 * lc + h) * 128, (2 * lc + h + 1) * 128)
             for lc in range(NLC)])
        other = np.concatenate(
            [np.arange((2 * lc + 1 - h) * 128, (2 * lc + 2 - h) * 128)
             for lc in range(NLC)])
        xTc = np.ascontiguousarray(
            x_in[b][np.concatenate([mine, other])].T).astype(ml_dtypes.bfloat16)
        rmask = (np.zeros((128, 128)) if h == 0
                 else np.full((128, 128), NEG)).astype(ml_dtypes.bfloat16)
        in_maps.append({
            "xT": xTc, "wq": wqt, "wk": wkt, "wv": wvt,
            "dmask": triW, "rmask": rmask,
            "hoff": np.array([[h * 1024, (1 - h) * 1024,
                               h * 512, (1 - h) * 512, h * 64]], np.uint32),
        })
    return in_maps


# out columns -> gpos position: A-half (cols 0..1023) covers positions
# [0-3, 8-11], B-half (cols 1024..2047) covers [4-7, 12-15].
_POS = [0, 1, 2, 3, 8, 9, 10, 11, 4, 5, 6, 7, 12, 13, 14, 15]


def assemble(results):
    z = np.empty((B, T, D), np.float32)
    for c in range(N_CORES):
        b, h = c // 2, c % 2
        o = np.asarray(results[c]["out"]).astype(np.float32)  # [64, 2048]
        for ci in range(16):
            p = _POS[ci]
            g = 2 * (p % 8) + (p // 8)
            z[b, g * 128:(g + 1) * 128, 64 * h:64 * h + 64] = \
                o[:, ci * 128:(ci + 1) * 128].T
    return z


def kernel(x_in, Wq, Wk, Wv):
    nc = _get_nc()
    in_maps = make_in_maps(x_in, Wq, Wk, Wv)
    res = run_bass_kernel_spmd(nc, in_maps, core_ids=list(range(N_CORES)))
    return assemble(res.results)


# revision 41
# speedup vs baseline: 1.0615x; 1.0615x over previous
"""Trainium2 Bass kernel for nn_AttentionHead (softmax over query axis).

Sharding (8 cores = 4 batches x 2): core pair (2b, 2b+1) handles batch b.
Rank h = c%2 owns KEY blocks of parity h: local chunk sb <-> global key
block gk = 2*sb + h.  Each core projects q for ALL 2048 rows (redundant,
avoids a mid-kernel AllGather) and k/v for its own 1024 rows only.

Per core (single SPMD program; h only appears in host-staged data):
  - host stages xT = x.T in bf16, columns [my 1024 rows | other 1024 rows]
    -> projections need no PE transposes and no gather
  - a tiny dummy collective issued at t~0 absorbs the collective-stream
    init barrier (~21.7us start + ~17us barrier + ~11.5us first-op cost)
    so the real collectives at the end pay only ~1.2us trigger latency
  - scores sT[s, t] = kb.T @ qT for queries t >= key block; causal via
    ADDITIVE masks on psum before exp (diag tri / first-remote-chunk)
  - Z[s] = sum_t E[s, t] is fully local (key-sharded) -> no AllReduce
  - v' = v/Z; AV accumulates zT[d, t] partials over local key blocks into
    4 PSUM banks; the low-column banks finish at sb=3 and ReduceScatter-A
    (bf16, pair, add) ships them while sb=4..7 still compute; RS-B ships
    the rest.  RS splits the D dim: core h ends with z[d in 64h..64h+64].
Host assembles the 8 [64, 2048] bf16 outputs into [4, 2048, 128] f32.
"""
import sys

for _p in ("/opt/trn_rl_repo",):
    if _p not in sys.path:
        sys.path.append(_p)

import numpy as np
import ml_dtypes

import concourse.bass as bass
import concourse.mybir as mybir
import concourse.tile as tile
from concourse import bacc
from concourse.bass import ds, ts
from concourse.bass_utils import run_bass_kernel_spmd
from concourse.masks import make_identity

BF16 = mybir.dt.bfloat16
F32 = mybir.dt.float32
U32 = mybir.dt.uint32
AF = mybir.ActivationFunctionType
ALU = mybir.AluOpType
AX = mybir.AxisListType

B, T, E, D = 4, 2048, 2048, 128
NLC = 8          # local 128-chunks per core (keys); queries = 2*NLC chunks
NE = 16          # E chunks of 128
SCALE = 1.0 / np.sqrt(D)
N_CORES = 8
REPLICA_GROUPS = [[0, 1], [2, 3], [4, 5], [6, 7]]
NEG = -1.0e30


def pieces(sb):
    """Column pieces [c0, width) of the valid query range [sb*128, 1024),
    split at absolute column 512 (PSUM-bank aligned)."""
    lo = sb * 128
    if lo < 512:
        return [(lo, 512 - lo), (512, 512)]
    return [(lo, 1024 - lo)]


def build_nc():
    nc = bacc.Bacc("TRN2", target_bir_lowering=False, debug=False,
                   num_devices=N_CORES)
    xT = nc.dram_tensor("xT", [E, T], BF16, kind="ExternalInput")
    wq = nc.dram_tensor("wq", [128, NE, D], BF16, kind="ExternalInput")
    wk = nc.dram_tensor("wk", [128, NE, D], BF16, kind="ExternalInput")
    wv = nc.dram_tensor("wv", [128, NE, D], BF16, kind="ExternalInput")
    dmask = nc.dram_tensor("dmask", [128, 128], BF16, kind="ExternalInput")
    rmask = nc.dram_tensor("rmask", [128, 128], BF16, kind="ExternalInput")
    hoff = nc.dram_tensor("hoff", [1, 5], U32, kind="ExternalInput")
    out = nc.dram_tensor("out", [64, T], BF16, kind="ExternalOutput")

    with tile.TileContext(nc) as tc:
        _body(nc, tc, xT, wq, wk, wv, dmask, rmask, hoff, out)
    nc.compile()
    return nc


def _body(nc, tc, xT, wq, wk, wv, dmask, rmask, hoff, out):
    with (
        tc.tile_pool(name="const", bufs=1) as const_pool,
        tc.tile_pool(name="dram", bufs=1, space="DRAM") as dram_pool,
        tc.tile_pool(name="proj", bufs=1) as proj_pool,
        tc.tile_pool(name="escore", bufs=1) as e_pool,
    ):
        # ---- constants / weights (SWDGE: keep HWDGE queues free for x) ----
        # ---- dummy early collective: absorbs cc-stream init + barrier ----
        dummy_sb = const_pool.tile([128, 16], BF16, name="dummy_sb")
        nc.vector.memset(dummy_sb[:], 0.0)
        dummy_in = dram_pool.tile([128, 16], BF16, name="dummy_in")
        dummy_out = dram_pool.tile([2, 128, 16], BF16, name="dummy_out")
        nc.gpsimd.dma_start(out=dummy_in[:], in_=dummy_sb[:])
        nc.gpsimd.collective_compute(
            "AllGather", ALU.bypass, replica_groups=REPLICA_GROUPS,
            ins=[dummy_in[:].opt()], outs=[dummy_out[:].opt()],
        )

        ident = const_pool.tile([128, 128], BF16, name="ident")
        make_identity(nc, ident)
        wq_sb = const_pool.tile([128, NE, D], BF16, name="wq_sb")
        wk_sb = const_pool.tile([128, NE, D], BF16, name="wk_sb")
        wv_sb = const_pool.tile([128, NE, D], BF16, name="wv_sb")
        nc.sync.dma_start(out=wk_sb[:], in_=wk[:])
        nc.scalar.dma_start(out=wq_sb[:], in_=wq[:])
        nc.gpsimd.dma_start(out=wv_sb[:], in_=wv[:])
        dmask_sb = const_pool.tile([128, 128], BF16, name="dmask_sb")
        rmask_sb = const_pool.tile([128, 128], BF16, name="rmask_sb")
        nc.gpsimd.dma_start(out=dmask_sb[:], in_=dmask[:])
        nc.gpsimd.dma_start(out=rmask_sb[:], in_=rmask[:])
        stats = const_pool.tile([128, NLC * 4], F32, name="stats")
        nc.vector.memset(stats[:], 0.0)

        # runtime pair-rank offsets: hoff = [h*1024, (1-h)*1024, h*512,
        # (1-h)*512, h*64]
        _r = [nc.sync.alloc_register(f"rv{i}") for i in range(5)]
        for i in range(5):
            nc.sync.reg_load(_r[i], hoff[0:1, i:i + 1])
        rv_h = nc.sync.snap(_r[0], donate=True, min_val=0, max_val=1024)
        rv_r = nc.sync.snap(_r[1], donate=True, min_val=0, max_val=1024)
        rv_h5 = nc.sync.snap(_r[2], donate=True, min_val=0, max_val=512)
        rv_r5 = nc.sync.snap(_r[3], donate=True, min_val=0, max_val=512)
        rv64 = nc.sync.snap(_r[4], donate=True, min_val=0, max_val=64)
        _r2 = nc.scalar.alloc_register("rv64b")
        nc.scalar.reg_load(_r2, hoff[0:1, 4:5])
        rv64b = nc.scalar.snap(_r2, donate=True, min_val=0, max_val=64)

        # ---- x load (pre-transposed bf16, fully contiguous, e-ordered) ----
        x_sb = []
        for e in range(NE):
            t_x = const_pool.tile([128, T], BF16, name=f"xT{e}")
            nc.sync.dma_start(out=t_x[:, ds(0, 1024)],
                              in_=xT[ts(e, 128), ds(0, 1024)])
            nc.scalar.dma_start(out=t_x[:, ds(1024, 1024)],
                                in_=xT[ts(e, 128), ds(1024, 1024)])
            x_sb.append(t_x)

        qT_sb = proj_pool.tile([128, T], BF16, name="qT_sb")
        kT_sb = proj_pool.tile([128, NLC * 128], BF16, name="kT_sb")
        vT_sb = proj_pool.tile([128, NLC * 128], BF16, name="vT_sb")
        v_loc = proj_pool.tile([128, NLC * 128], BF16, name="v_loc")
        v_sc = proj_pool.tile([128, NLC * 128], BF16, name="v_sc")
        recip = const_pool.tile([128, NLC], F32, name="recip")

        # ---- k & q projections, interleaved per e-chunk ----
        with tc.tile_pool(name="pj_psum", bufs=1, space="PSUM") as pj:
            junk = pj.tile([128, 512], F32, tag="k0")
            for i in range(6):
                nc.tensor.matmul(junk[:], lhsT=wk_sb[:, i, :],
                                 rhs=wk_sb[:, ds(0, 4), :],
                                 start=True, stop=True)
            qp = [pj.tile([128, 512], F32, tag=f"p{i}", name=f"qp{i}")
                  for i in range(4)]
            k0 = pj.tile([128, 512], F32, tag="k0")
            k1 = pj.tile([128, 512], F32, tag="k1")
            for e in range(NE):
                st, sp = (e == 0), (e == NE - 1)
                nc.tensor.matmul(k0[:], lhsT=wk_sb[:, e, :],
                                 rhs=x_sb[e][:, ds(0, 512)], start=st, stop=sp)
                nc.tensor.matmul(k1[:], lhsT=wk_sb[:, e, :],
                                 rhs=x_sb[e][:, ds(512, 512)], start=st, stop=sp)
                for i in range(4):
                    nc.tensor.matmul(qp[i][:], lhsT=wq_sb[:, e, :],
                                     rhs=x_sb[e][:, ds(i * 512, 512)],
                                     start=st, stop=sp)
            nc.vector.tensor_copy(out=kT_sb[:, ds(0, 512)], in_=k0[:])
            nc.vector.tensor_copy(out=qT_sb[:, ds(0, 512)], in_=qp[0][:])
            nc.vector.tensor_copy(out=kT_sb[:, ds(512, 512)], in_=k1[:])
            for i in range(1, 4):
                nc.vector.tensor_copy(out=qT_sb[:, ds(i * 512, 512)], in_=qp[i][:])

        # ---- scores / exp / piece-sums; masks ride the PE as matmuls ----
        with (
            tc.tile_pool(name="vp_psum", bufs=1, space="PSUM") as vp,
            tc.tile_pool(name="sc_psum", bufs=2, space="PSUM") as scp,
            tc.tile_pool(name="zt_psum", bufs=1, space="PSUM") as ztp,
        ):
            # ---- v projection + v natural via PE transpose ----
            v0 = vp.tile([128, 512], F32, tag="v0")
            v1 = vp.tile([128, 512], F32, tag="v1")
            for e in range(NE):
                st, sp = (e == 0), (e == NE - 1)
                nc.tensor.matmul(v0[:], lhsT=wv_sb[:, e, :],
                                 rhs=x_sb[e][:, ds(0, 512)], start=st, stop=sp)
                nc.tensor.matmul(v1[:], lhsT=wv_sb[:, e, :],
                                 rhs=x_sb[e][:, ds(512, 512)], start=st, stop=sp)
            nc.vector.tensor_copy(out=vT_sb[:, ds(0, 512)], in_=v0[:])
            nc.vector.tensor_copy(out=vT_sb[:, ds(512, 512)], in_=v1[:])
            for sb in range(NLC):
                tpv = vp.tile([128, 128], BF16, tag="v0")
                nc.tensor.transpose(out=tpv[:], in_=vT_sb[:, ts(sb, 128)],
                                    identity=ident[:])
                nc.vector.tensor_copy(out=v_loc[:, ts(sb, 128)], in_=tpv[:])

            # ---- scores / exp / Z / v' / AV streamed per key block ----
            zt = [ztp.tile([128, 512], F32, tag=f"zt{j}", name=f"zt{j}")
                  for j in range(4)]
            zT_f = proj_pool.tile([128, 2048], BF16, name="zT_f")
            zinA = dram_pool.tile([128, 1024], BF16, name="zinA")
            zinB = dram_pool.tile([128, 1024], BF16, name="zinB")
            agA = dram_pool.tile([2, 128, 1024], BF16, name="agA")
            agB = dram_pool.tile([2, 128, 1024], BF16, name="agB")
            pA0 = proj_pool.tile([64, 1024], BF16, name="pA0")
            pA1 = proj_pool.tile([64, 1024], BF16, name="pA1")
            pB0 = proj_pool.tile([64, 1024], BF16, name="pB0")
            pB1 = proj_pool.tile([64, 1024], BF16, name="pB1")
            zoA = proj_pool.tile([64, 1024], BF16, name="zoA")
            zoB = proj_pool.tile([64, 1024], BF16, name="zoB")
            e_tiles = []

            def do_av(sb):
                esb = e_tiles[sb]
                zs = const_pool.tile([128, 1], F32, tag="zs", name="zs")
                nc.vector.reduce_sum(out=zs[:], in_=stats[:, ds(sb * 4, 4)],
                                     axis=AX.X)
                nc.vector.reciprocal(out=recip[:, ds(sb, 1)], in_=zs[:])
                nc.vector.tensor_scalar_mul(
                    out=v_sc[:, ts(sb, 128)], in0=v_loc[:, ts(sb, 128)],
                    scalar1=recip[:, ds(sb, 1)])
                vs = v_sc[:, ts(sb, 128)]
                for half in range(2):
                    for (c0, pw) in pieces(sb):
                        j = half * 2 + (c0 // 512)
                        nc.tensor.matmul(
                            zt[j][:, ds(c0 % 512, pw)], lhsT=vs,
                            rhs=esb[:, ds(half * 1024 + c0, pw)],
                            start=(sb == 0),
                            stop=(sb == (3 if c0 < 512 else NLC - 1)),
                            skip_group_check=True)
                if sb == 3:
                    # low-column banks are complete: ship the first half now
                    nc.vector.tensor_copy(out=zT_f[:, ds(0, 512)], in_=zt[0][:])
                    nc.vector.tensor_copy(out=zT_f[:, ds(1024, 512)],
                                          in_=zt[2][:])
                    nc.sync.dma_start(out=zinA[:, ds(rv_h5, 512)],
                                      in_=zT_f[:, ds(0, 512)])
                    nc.sync.dma_start(out=zinA[:, ds(rv_r5, 512)],
                                      in_=zT_f[:, ds(1024, 512)])
                    nc.gpsimd.collective_compute(
                        "AllGather", ALU.bypass, replica_groups=REPLICA_GROUPS,
                        ins=[zinA[:].opt()], outs=[agA[:].opt()],
                    )
                    nc.sync.dma_start(out=pA0[:],
                                      in_=agA[0, ds(rv64, 64), :])
                    nc.scalar.dma_start(out=pA1[:],
                                        in_=agA[1, ds(rv64b, 64), :])
                    nc.vector.tensor_tensor(out=zoA[:], in0=pA0[:],
                                            in1=pA1[:], op=ALU.add)
                    nc.sync.dma_start(out=out[:, ds(0, 1024)], in_=zoA[:])

            for sb in range(NLC):
                esb = e_pool.tile([128, 2048], BF16, name=f"esb{sb}")
                e_tiles.append(esb)
                kb = kT_sb[:, ts(sb, 128)]
                for half in range(2):  # 0: my queries, 1: peer's queries
                    mask = dmask_sb if half == 0 else rmask_sb
                    for pi, (c0, pw) in enumerate(pieces(sb)):
                        sc = scp.tile([128, 512], F32, tag="sc")
                        nc.tensor.matmul(sc[:, ds(0, pw)], lhsT=kb,
                                         rhs=qT_sb[:, ds(half * 1024 + c0, pw)],
                                         start=True, stop=(pi != 0),
                                         skip_group_check=True)
                        if pi == 0:  # causal-boundary chunk: additive mask
                            # sc[s,t] += mask[t,s] (= mask.T @ I), on PE
                            nc.tensor.matmul(sc[:, ds(0, 128)], lhsT=mask[:],
                                             rhs=ident[:],
                                             start=False, stop=True,
                                             skip_group_check=True)
                        nc.scalar.activation(out=esb[:, ds(half * 1024 + c0, pw)],
                                             in_=sc[:, ds(0, pw)],
                                             func=AF.Exp, scale=SCALE)
                        nc.vector.reduce_sum(
                            out=stats[:, ds(sb * 4 + half * 2 + pi, 1)],
                            in_=esb[:, ds(half * 1024 + c0, pw)], axis=AX.X)
                # AV trails the scores by 2 key blocks so the exp/Z chain
                # never head-of-line-blocks the PE queue
                if sb >= 2:
                    do_av(sb - 2)
            do_av(NLC - 2)
            do_av(NLC - 1)
            nc.vector.tensor_copy(out=zT_f[:, ds(512, 512)], in_=zt[1][:])
            nc.vector.tensor_copy(out=zT_f[:, ds(1536, 512)], in_=zt[3][:])
            nc.sync.dma_start(out=zinB[:, ds(rv_h5, 512)],
                              in_=zT_f[:, ds(512, 512)])
            nc.sync.dma_start(out=zinB[:, ds(rv_r5, 512)],
                              in_=zT_f[:, ds(1536, 512)])
            nc.gpsimd.collective_compute(
                "AllGather", ALU.bypass, replica_groups=REPLICA_GROUPS,
                ins=[zinB[:].opt()], outs=[agB[:].opt()],
            )
            nc.sync.dma_start(out=pB0[:], in_=agB[0, ds(rv64, 64), :])
            nc.scalar.dma_start(out=pB1[:], in_=agB[1, ds(rv64b, 64), :])
            nc.vector.tensor_tensor(out=zoB[:], in0=pB0[:], in1=pB1[:],
                                    op=ALU.add)
            nc.sync.dma_start(out=out[:, ds(1024, 1024)], in_=zoB[:])


_NC_CACHE = None


def _get_nc():
    global _NC_CACHE
    if _NC_CACHE is None:
        _NC_CACHE = build_nc()
    return _NC_CACHE


def _w_tiles(W):
    return np.ascontiguousarray(
        np.asarray(W, np.float32).reshape(NE, 128, D).transpose(1, 0, 2)
    ).astype(ml_dtypes.bfloat16)


def make_in_maps(x_in, Wq, Wk, Wv):
    x_in = np.asarray(x_in, np.float32)
    wqt, wkt, wvt = _w_tiles(Wq), _w_tiles(Wk), _w_tiles(Wv)
    # mask weights: score psum gets += maskW[t, s] via maskW.T @ I, so
    # maskW[e, s] = NEG where s > e adds NEG at [s, t] for t < s (diag),
    # and all-NEG adds NEG everywhere (invalid first remote chunk, h=1)
    triW = np.where(np.arange(128)[:, None] < np.arange(128)[None, :],
                    NEG, 0.0).astype(ml_dtypes.bfloat16)
    in_maps = []
    for c in range(N_CORES):
        b, h = c // 2, c % 2
        mine = np.concatenate(
            [np.arange((2 * lc + h) * 128, (2 * lc + h + 1) * 128)
             for lc in range(NLC)])
        other = np.concatenate(
            [np.arange((2 * lc + 1 - h) * 128, (2 * lc + 2 - h) * 128)
             for lc in range(NLC)])
        xTc = np.ascontiguousarray(
            x_in[b][np.concatenate([mine, other])].T).astype(ml_dtypes.bfloat16)
        rmask = (np.zeros((128, 128)) if h == 0
                 else np.full((128, 128), NEG)).astype(ml_dtypes.bfloat16)
        in_maps.append({
            "xT": xTc, "wq": wqt, "wk": wkt, "wv": wvt,
            "dmask": triW, "rmask": rmask,
            "hoff": np.array([[h * 1024, (1 - h) * 1024,
                               h * 512, (1 - h) * 512, h * 64]], np.uint32),
        })
    return in_maps


# out columns -> gpos position: A-half (cols 0..1023) covers positions
# [0-3, 8-11], B-half (cols 1024..2047) covers [4-7, 12-15].
_POS = [0, 1, 2, 3, 8, 9, 10, 11, 4, 5, 6, 7, 12, 13, 14, 15]


def assemble(results):
    z = np.empty((B, T, D), np.float32)
    for c in range(N_CORES):
        b, h = c // 2, c % 2
        o = np.asarray(results[c]["out"]).astype(np.float32)  # [64, 2048]
        for ci in range(16):
            p = _POS[ci]
            g = 2 * (p % 8) + (p // 8)
            z[b, g * 128:(g + 1) * 128, 64 * h:64 * h + 64] = \
                o[:, ci * 128:(ci + 1) * 128].T
    return z


def kernel(x_in, Wq, Wk, Wv):
    nc = _get_nc()
    in_maps = make_in_maps(x_in, Wq, Wk, Wv)
    res = run_bass_kernel_spmd(nc, in_maps, core_ids=list(range(N_CORES)))
    return assemble(res.results)
